# revision 19
# baseline (speedup 1.0000x reference)
"""Trainium2 Bass kernel for sliding-window causal attention block.

Reference computation (per batch b):
  qh = (q @ wq.T)  -> [S, H, Dh], RoPE'd; kh likewise; vh = v @ wv.T
  scores = qh . kh / sqrt(Dh), sliding-window causal (j in (i-512, i])
  out = softmax(scores) @ vh  -> [S, H*Dh] @ wo.T -> [S, D]

Sharding: 8 cores = 2 batches x 4 head-groups (4 heads each).
Each core computes y_part[b] = attn(heads g) @ wo[:, g].T  (f32 partial);
host sums the 4 partials per batch and casts to f16.

Layout strategy per core (everything hardcoded for S=2048, D=1024,
Hc=4 heads, Dh=64, WINDOW=512):
  - host passes x.T [D, S] and head-sliced weights pre-transposed, so all
    matmuls use natural [K-on-partition] tiles with contiguous DMA.
  - q,k projections computed transposed: qT/kT [e, s] (e = head-major,
    RoPE-permuted so even/odd rotary halves are contiguous partition
    blocks); RoPE applied as q*A + shuffle(q)*B where shuffle is a
    partition-swap done on the TensorEngine with a permutation matmul.
  - scores computed transposed per (q-tile t, head h): S.T [j, i] chunks
    via lhsT=kT, rhs=qT (K=Dh=64).  Sliding window -> only 5 key chunks.
  - window masks added on PSUM (additive -30000), exp on ScalarE with the
    1/sqrt(Dh) scale folded in; no max-subtraction (|score|/8 <= 8 by
    Cauchy-Schwarz so exp <= e^8, safely inside f16/f32 range).
  - PV: out[i, dh] via lhsT = pT chunk [j, i], rhs = v_ext [j, 65] whose
    last column of ones yields the softmax row-sum for free.
  - normalize + f32->f16 cast fused into one ScalarE copy with
    scale=reciprocal(row-sum) per partition.
  - attn [i, e] transposed (PE) to attnT [e, i] for the output projection
    y[s, :] += attnT.T @ woT, accumulated over e-chunks in PSUM, then
    DMA'd straight from PSUM to DRAM as f32 partials.
"""

import os
import sys

import numpy as np

for _p in ("/opt/trn_rl_repo", "/root/.axon_site/_ro/trn_rl_repo"):
    if os.path.isdir(_p) and _p not in sys.path:
        sys.path.insert(0, _p)

DIM = 1024
NUM_HEADS = 16
HEAD_DIM = 64
WINDOW = 512
S = 2048
B = 2
HPC = 4  # heads per core
E = HPC * HEAD_DIM  # 256 = per-core hidden slice
N_CORES = 8
ST = S // 128  # 16 query tiles of 128
KC = DIM // 128  # 8 contraction chunks for projections
NEG = -30000.0


def _rope_tables():
    # A/B factor tables in the RoPE-permuted [p, s] layout, f32.
    f = np.arange(32, dtype=np.float64)
    inv_freq = 1.0 / (10000.0 ** (2.0 * f / HEAD_DIM))  # [32]
    ang = np.arange(S, dtype=np.float64)[None, :] * inv_freq[:, None]  # [32, S]
    cos = np.cos(ang)
    sin = np.sin(ang)
    A = np.empty((128, S), dtype=np.float32)
    Bt = np.empty((128, S), dtype=np.float32)
    for blk in range(2):  # two 64-partition head blocks per tile
        o = blk * 64
        A[o : o + 32] = cos
        A[o + 32 : o + 64] = cos
        Bt[o : o + 32] = -sin
        Bt[o + 32 : o + 64] = sin
    return A, Bt


def _consts():
    A, Bt = _rope_tables()
    j = np.arange(128)[:, None]
    i = np.arange(128)[None, :]
    maskD = np.where(i >= j, 0.0, NEG).astype(np.float32)  # diagonal chunk
    maskL = np.where(j > i, 0.0, NEG).astype(np.float32)  # leftmost chunk
    permM = np.zeros((128, 128), dtype=np.float16)
    for m in range(128):
        partner = m + 32 if (m % 64) < 32 else m - 32
        permM[partner, m] = 1.0
    ident = np.eye(128, dtype=np.float16)
    return {
        "ropeA": A.astype(np.float16),
        "ropeB": Bt.astype(np.float16),
        "maskD": maskD,
        "maskL": maskL,
        "permM": permM,
        "ident": ident,
    }


def _head_perm():
    # within each head: evens then odds
    p = np.empty(E, dtype=np.int64)
    for h in range(HPC):
        base = h * HEAD_DIM
        p[base : base + 32] = base + np.arange(0, 64, 2)
        p[base + 32 : base + 64] = base + np.arange(1, 64, 2)
    return p


def build_bass(do_compile=True):
    import concourse.bacc as bacc
    import concourse.mybir as mybir
    import concourse.tile as tile

    f16 = mybir.dt.float16
    f32 = mybir.dt.float32
    Exp = mybir.ActivationFunctionType.Exp

    nc = bacc.Bacc("TRN2")

    bf16 = mybir.dt.bfloat16
    xqT = nc.dram_tensor("xqT", [DIM, S], f16, kind="ExternalInput")
    xkT = nc.dram_tensor("xkT", [DIM, S], f16, kind="ExternalInput")
    xvT = nc.dram_tensor("xvT", [DIM, S], f16, kind="ExternalInput")
    wqT = nc.dram_tensor("wqT", [DIM, E], f16, kind="ExternalInput")
    wkT = nc.dram_tensor("wkT", [DIM, E], f16, kind="ExternalInput")
    wvT = nc.dram_tensor("wvT", [DIM, E], f16, kind="ExternalInput")
    woT = nc.dram_tensor("woT", [E, DIM], f16, kind="ExternalInput")
    ropeA = nc.dram_tensor("ropeA", [128, S], f16, kind="ExternalInput")
    ropeB = nc.dram_tensor("ropeB", [128, S], f16, kind="ExternalInput")
    maskD = nc.dram_tensor("maskD", [128, 128], f32, kind="ExternalInput")
    maskL = nc.dram_tensor("maskL", [128, 128], f32, kind="ExternalInput")
    permM = nc.dram_tensor("permM", [128, 128], f16, kind="ExternalInput")
    ident = nc.dram_tensor("ident", [128, 128], f16, kind="ExternalInput")
    y = nc.dram_tensor("y", [S, DIM], bf16, kind="ExternalOutput")

    with tile.TileContext(nc) as tc:
        # All pools stay open for the whole kernel: SBUF/PSUM memory is never
        # reused across phases, so no instruction inherits pool-release
        # dependencies (DVE TensorTensor only supports 2 sync waits and the
        # release fan-in of a recycled slot can reach 8+ DMA-queue sems).
        with tc.tile_pool(name="res", bufs=1) as res, \
             tc.tile_pool(name="xp", bufs=3) as xp, \
             tc.tile_pool(name="tmp", bufs=3) as tmpp, \
             tc.tile_pool(name="sb2", bufs=2) as sb2:
            # resident tensors
            qT = res.tile([128, 2, S], f16)
            kT = res.tile([128, 2, S], f16)
            v_sb = res.tile([128, ST, HPC, 65], f16)
            woT_sb = res.tile([128, 2, DIM], f16)
            maskD_in = res.tile([128, 128], f32)
            maskL_in = res.tile([128, 128], f32)
            maskD_sb = res.tile([128, 128], f32)
            maskL_sb = res.tile([128, 128], f32)
            ident_sb = res.tile([128, 128], f16)
            wq_sb = res.tile([128, KC, E], f16)
            wk_sb = res.tile([128, KC, E], f16)
            wv_sb = res.tile([128, KC, E], f16)
            A_sb = res.tile([128, S], f16)
            B_sb = res.tile([128, S], f16)
            perm_sb = res.tile([128, 128], f16)
            qraw = res.tile([128, 2, S], f16)
            kraw = res.tile([128, 2, S], f16)

            nc.sync.dma_start(out=woT_sb, in_=woT[:].rearrange("(c p) n -> p c n", p=128))
            nc.sync.dma_start(out=maskD_in, in_=maskD[:])
            nc.sync.dma_start(out=maskL_in, in_=maskL[:])
            nc.sync.dma_start(out=ident_sb, in_=ident[:])
            # launder the masks through DVE so the per-tile mask adds depend
            # on the DVE engine (program order, no sem) instead of DMA sems
            nc.vector.tensor_copy(maskD_sb, maskD_in)
            nc.vector.tensor_copy(maskL_sb, maskL_in)
            nc.any.memset(v_sb[:, :, :, 64:65], 1.0)

            for dram, sb in ((wqT, wq_sb), (wkT, wk_sb), (wvT, wv_sb)):
                nc.sync.dma_start(out=sb, in_=dram[:].rearrange("(c p) e -> p c e", p=128))
            nc.sync.dma_start(out=A_sb, in_=ropeA[:])
            nc.sync.dma_start(out=B_sb, in_=ropeB[:])
            nc.sync.dma_start(out=perm_sb, in_=permM[:])

            # ------------- phase 1: projections + RoPE, streamed by s -------------
            with tc.tile_pool(name="pp", bufs=2, space="PSUM") as pp:
              for sc in range(4):
                ssl = slice(sc * 512, (sc + 1) * 512)
                # q/k projections, transposed output [e, s]
                for dram, w_sb, raw in ((xqT, wq_sb, qraw), (xkT, wk_sb, kraw)):
                    xt = xp.tile([128, KC, 512], f16, tag="xt")
                    for kc in range(KC):
                        nc.sync.dma_start(
                            out=xt[:, kc, :], in_=dram[kc * 128 : (kc + 1) * 128, ssl]
                        )
                    for ec in range(2):
                        ps = pp.tile([128, 512], f32, tag="pp")
                        for kc in range(KC):
                            nc.tensor.matmul(
                                ps,
                                lhsT=w_sb[:, kc, ec * 128 : (ec + 1) * 128],
                                rhs=xt[:, kc, :],
                                start=(kc == 0),
                                stop=(kc == KC - 1),
                            )
                        nc.scalar.copy(raw[:, ec, ssl], ps)
                # v projection, natural output [s, e], into v_ext slots
                xt = xp.tile([128, KC, 512], f16, tag="xt")
                for kc in range(KC):
                    nc.sync.dma_start(
                        out=xt[:, kc, :], in_=xvT[kc * 128 : (kc + 1) * 128, ssl]
                    )
                for st4 in range(4):
                    sc16 = sc * 4 + st4
                    ps = pp.tile([128, E], f32, tag="ppv")
                    for kc in range(KC):
                        nc.tensor.matmul(
                            ps,
                            lhsT=xt[:, kc, st4 * 128 : (st4 + 1) * 128],
                            rhs=wv_sb[:, kc, :],
                            start=(kc == 0),
                            stop=(kc == KC - 1),
                        )
                    nc.scalar.copy(
                        v_sb[:, sc16, :, 0:64],
                        ps.rearrange("p (h d) -> p h d", h=HPC),
                    )
                # RoPE on this s-chunk: out = raw*A + permute(raw)*B
                for raw, out_sb in ((qraw, qT), (kraw, kT)):
                    for ec in range(2):
                        psh = pp.tile([128, 512], f32, tag="perm")
                        nc.tensor.matmul(
                            psh, lhsT=perm_sb, rhs=raw[:, ec, ssl],
                            start=True, stop=True,
                        )
                        t1 = tmpp.tile([128, 512], f16, tag="t1")
                        nc.vector.tensor_mul(t1, raw[:, ec, ssl], A_sb[:, ssl])
                        t2 = tmpp.tile([128, 512], f16, tag="t2")
                        nc.vector.tensor_mul(t2, psh, B_sb[:, ssl])
                        nc.vector.tensor_add(out_sb[:, ec, ssl], t1, t2)

            # ---------------- phase 2: attention + out-proj ----------------
            with tc.tile_pool(name="pst", bufs=2, space="PSUM") as stp, \
                 tc.tile_pool(name="po", bufs=2, space="PSUM") as op, \
                 tc.tile_pool(name="ptr", bufs=1, space="PSUM") as trp, \
                 tc.tile_pool(name="py", bufs=1, space="PSUM") as yp:
                for t in range(ST):
                    c0 = max(0, t - 4)
                    ncv = t - c0 + 1
                    tsl = slice(t * 128, (t + 1) * 128)
                    attn_t = sb2.tile([128, HPC, 64], f16, tag="attn")
                    po = op.tile([128, HPC, 66], f32, tag="po")
                    for h in range(HPC):
                        ec, hh = h // 2, h % 2
                        psl = slice(hh * 64, (hh + 1) * 64)
                        pst = stp.tile([128, 5, 128], f32, tag="st")
                        for si, c in enumerate(range(c0, t + 1)):
                            nc.tensor.matmul(
                                pst[:, si, :],
                                lhsT=kT[psl, ec, c * 128 : (c + 1) * 128],
                                rhs=qT[psl, ec, tsl],
                                start=True,
                                stop=True,
                            )
                        if t >= 4:
                            nc.vector.tensor_add(pst[:, 0, :], pst[:, 0, :], maskL_sb)
                        nc.vector.tensor_add(
                            pst[:, ncv - 1, :], pst[:, ncv - 1, :], maskD_sb
                        )
                        pt = sb2.tile([128, 5, 128], f16, tag="pt", bufs=4)
                        nc.scalar.activation(
                            pt[:, 0:ncv, :], pst[:, 0:ncv, :], Exp, scale=0.125
                        )
                        for si, c in enumerate(range(c0, t + 1)):
                            nc.tensor.matmul(
                                po[:, h, 0:65],
                                lhsT=pt[:, si, :],
                                rhs=v_sb[:, c, h, :],
                                start=(si == 0),
                                stop=(si == ncv - 1),
                            )
                    rc = sb2.tile([128, HPC, 1], f32, tag="rc")
                    nc.vector.reciprocal(rc, po[:, :, 64:65])
                    nc.vector.tensor_mul(
                        attn_t, po[:, :, 0:64], rc.broadcast_to([128, HPC, 64])
                    )

                    attnT_t = sb2.tile([128, 2, 128], f16, tag="attnT")
                    attn_flat = attn_t.rearrange("p h d -> p (h d)")
                    for ec in range(2):
                        ptr = trp.tile([128, 128], f16, tag="tr")
                        nc.tensor.transpose(
                            ptr, attn_flat[:, ec * 128 : (ec + 1) * 128], ident_sb
                        )
                        nc.vector.tensor_copy(attnT_t[:, ec, :], ptr)
                    for nch in range(2):
                        py = yp.tile([128, 512], f32, tag="py")
                        for ec in range(2):
                            nc.tensor.matmul(
                                py,
                                lhsT=attnT_t[:, ec, :],
                                rhs=woT_sb[:, ec, nch * 512 : (nch + 1) * 512],
                                start=(ec == 0),
                                stop=(ec == 1),
                            )
                        y_sb = sb2.tile([128, 512], bf16, tag="ysb", bufs=3)
                        # alternate the PSUM->SBUF cast between ACT and DVE to
                        # balance engine load
                        if (t + nch) % 2 == 0:
                            nc.scalar.copy(y_sb, py)
                        else:
                            nc.vector.tensor_copy(y_sb, py)
                        nc.sync.dma_start(
                            out=y[tsl, nch * 512 : (nch + 1) * 512], in_=y_sb
                        )
    if do_compile:
        # Bacc pass pipeline: splits multi-sem waits into EventSemaphores
        # (HW allows 1 sync wait per instruction), register allocation, DCE.
        nc.compile()
    return nc


_CACHE = {}


def _get_nc():
    if "nc" not in _CACHE:
        _CACHE["nc"] = build_bass()
    return _CACHE["nc"]


def _in_maps(q, k, v, wq, wk, wv, wo):
    consts = _consts()
    perm = _head_perm()
    maps = []
    for c in range(N_CORES):
        b, g = c // 4, c % 4
        esl = slice(g * E, (g + 1) * E)
        wq_c = wq[esl][perm]
        wk_c = wk[esl][perm]
        m = {
            "xqT": np.ascontiguousarray(q[b].T),
            "xkT": np.ascontiguousarray(k[b].T),
            "xvT": np.ascontiguousarray(v[b].T),
            "wqT": np.ascontiguousarray(wq_c.T),
            "wkT": np.ascontiguousarray(wk_c.T),
            "wvT": np.ascontiguousarray(wv[esl].T),
            "woT": np.ascontiguousarray(wo[:, esl].T),
        }
        m.update(consts)
        maps.append(m)
    return maps


def kernel(q, k, v, wq, wk, wv, wo):
    q, k, v = (np.asarray(a, dtype=np.float16) for a in (q, k, v))
    wq, wk, wv, wo = (np.asarray(a, dtype=np.float16) for a in (wq, wk, wv, wo))
    from concourse.bass_utils import run_bass_kernel_spmd

    nc = _get_nc()
    maps = _in_maps(q, k, v, wq, wk, wv, wo)
    res = run_bass_kernel_spmd(nc, maps, core_ids=list(range(N_CORES)))
    out = np.zeros((B, S, DIM), dtype=np.float32)
    for c in range(N_CORES):
        out[c // 4] += np.asarray(res.results[c]["y"]).astype(np.float32)
    return out.astype(np.float16)


# revision 28
# speedup vs baseline: 1.0619x; 1.0619x over previous
"""Trainium2 Bass kernel for sliding-window causal attention block.

Reference computation (per batch b):
  qh = (q @ wq.T)  -> [S, H, Dh], RoPE'd; kh likewise; vh = v @ wv.T
  scores = qh . kh / sqrt(Dh), sliding-window causal (j in (i-512, i])
  out = softmax(scores) @ vh  -> [S, H*Dh] @ wo.T -> [S, D]

Sharding: 8 cores = 2 batches x 4 head-groups (4 heads each).
Each core computes y_part[b] = attn(heads g) @ wo[:, g].T  (f32 partial);
host sums the 4 partials per batch and casts to f16.

Layout strategy per core (everything hardcoded for S=2048, D=1024,
Hc=4 heads, Dh=64, WINDOW=512):
  - host passes x.T [D, S] and head-sliced weights pre-transposed, so all
    matmuls use natural [K-on-partition] tiles with contiguous DMA.
  - q,k projections computed transposed: qT/kT [e, s] (e = head-major,
    RoPE-permuted so even/odd rotary halves are contiguous partition
    blocks); RoPE applied as q*A + shuffle(q)*B where shuffle is a
    partition-swap done on the TensorEngine with a permutation matmul.
  - scores computed transposed per (q-tile t, head h): S.T [j, i] chunks
    via lhsT=kT, rhs=qT (K=Dh=64).  Sliding window -> only 5 key chunks.
  - window masks added on PSUM (additive -30000), exp on ScalarE with the
    1/sqrt(Dh) scale folded in; no max-subtraction (|score|/8 <= 8 by
    Cauchy-Schwarz so exp <= e^8, safely inside f16/f32 range).
  - PV: out[i, dh] via lhsT = pT chunk [j, i], rhs = v_ext [j, 65] whose
    last column of ones yields the softmax row-sum for free; all four
    heads accumulate into one per-tile PSUM [128, 4, 66].
  - normalize: one DVE reciprocal of the row-sums plus one broadcast
    multiply per q-tile.
  - attn [i, e] transposed (PE) to attnT [e, i] for the output projection
    y[s, :] += attnT.T @ woT, accumulated over e-chunks in PSUM, cast to
    bf16 (alternating ScalarE/VectorE to balance load) and DMA'd out;
    the host sums the four bf16 partials per batch in f32.

Engine budget per core (cost model): PE ~84us (projections at the f16
roofline + QK/PV), ACT ~69us (exp), DVE ~64us (masks/normalize/copies),
DMA ~75us, GpSimd ~7us (RoPE multiply); simulated makespan ~132us.
Boundary-window masks are applied as ONE strided-AP add per (tile, head)
covering both boundary chunks; the SBUF-only RoPE multiply runs on the
otherwise-idle GpSimd engine.
"""

import os
import sys

import numpy as np

for _p in ("/opt/trn_rl_repo", "/root/.axon_site/_ro/trn_rl_repo"):
    if os.path.isdir(_p) and _p not in sys.path:
        sys.path.insert(0, _p)

DIM = 1024
NUM_HEADS = 16
HEAD_DIM = 64
WINDOW = 512
S = 2048
B = 2
HPC = 4  # heads per core
E = HPC * HEAD_DIM  # 256 = per-core hidden slice
N_CORES = 8
ST = S // 128  # 16 query tiles of 128
KC = DIM // 128  # 8 contraction chunks for projections
NEG = -30000.0


def _rope_tables():
    # A/B factor tables in the RoPE-permuted [p, s] layout, f32.
    f = np.arange(32, dtype=np.float64)
    inv_freq = 1.0 / (10000.0 ** (2.0 * f / HEAD_DIM))  # [32]
    ang = np.arange(S, dtype=np.float64)[None, :] * inv_freq[:, None]  # [32, S]
    cos = np.cos(ang)
    sin = np.sin(ang)
    A = np.empty((128, S), dtype=np.float32)
    Bt = np.empty((128, S), dtype=np.float32)
    for blk in range(2):  # two 64-partition head blocks per tile
        o = blk * 64
        A[o : o + 32] = cos
        A[o + 32 : o + 64] = cos
        Bt[o : o + 32] = -sin
        Bt[o + 32 : o + 64] = sin
    return A, Bt


def _consts():
    A, Bt = _rope_tables()
    j = np.arange(128)[:, None]
    i = np.arange(128)[None, :]
    maskD = np.where(i >= j, 0.0, NEG).astype(np.float32)  # diagonal chunk
    maskL = np.where(j > i, 0.0, NEG).astype(np.float32)  # leftmost chunk
    permM = np.zeros((128, 128), dtype=np.float16)
    for m in range(128):
        partner = m + 32 if (m % 64) < 32 else m - 32
        permM[partner, m] = 1.0
    ident = np.eye(128, dtype=np.float16)
    return {
        "ropeA": A.astype(np.float16),
        "ropeB": Bt.astype(np.float16),
        "maskD": maskD,
        "maskL": maskL,
        "permM": permM,
        "ident": ident,
    }


def _head_perm():
    # within each head: evens then odds
    p = np.empty(E, dtype=np.int64)
    for h in range(HPC):
        base = h * HEAD_DIM
        p[base : base + 32] = base + np.arange(0, 64, 2)
        p[base + 32 : base + 64] = base + np.arange(1, 64, 2)
    return p


def build_bass(do_compile=True):
    import concourse.bacc as bacc
    import concourse.mybir as mybir
    import concourse.tile as tile

    f16 = mybir.dt.float16
    f32 = mybir.dt.float32
    Exp = mybir.ActivationFunctionType.Exp

    nc = bacc.Bacc("TRN2")

    bf16 = mybir.dt.bfloat16
    xqT = nc.dram_tensor("xqT", [DIM, S], f16, kind="ExternalInput")
    xkT = nc.dram_tensor("xkT", [DIM, S], f16, kind="ExternalInput")
    xvT = nc.dram_tensor("xvT", [DIM, S], f16, kind="ExternalInput")
    wqT = nc.dram_tensor("wqT", [DIM, E], f16, kind="ExternalInput")
    wkT = nc.dram_tensor("wkT", [DIM, E], f16, kind="ExternalInput")
    wvT = nc.dram_tensor("wvT", [DIM, E], f16, kind="ExternalInput")
    woT = nc.dram_tensor("woT", [E, DIM], f16, kind="ExternalInput")
    ropeA = nc.dram_tensor("ropeA", [128, S], f16, kind="ExternalInput")
    ropeB = nc.dram_tensor("ropeB", [128, S], f16, kind="ExternalInput")
    maskD = nc.dram_tensor("maskD", [128, 128], f32, kind="ExternalInput")
    maskL = nc.dram_tensor("maskL", [128, 128], f32, kind="ExternalInput")
    permM = nc.dram_tensor("permM", [128, 128], f16, kind="ExternalInput")
    ident = nc.dram_tensor("ident", [128, 128], f16, kind="ExternalInput")
    y = nc.dram_tensor("y", [S, DIM], bf16, kind="ExternalOutput")

    with tile.TileContext(nc) as tc:
        # All pools stay open for the whole kernel: SBUF/PSUM memory is never
        # reused across phases, so no instruction inherits pool-release
        # dependencies (DVE TensorTensor only supports 2 sync waits and the
        # release fan-in of a recycled slot can reach 8+ DMA-queue sems).
        with tc.tile_pool(name="res", bufs=1) as res, \
             tc.tile_pool(name="xp", bufs=3) as xp, \
             tc.tile_pool(name="tmp", bufs=3) as tmpp, \
             tc.tile_pool(name="sb2", bufs=2) as sb2:
            # resident tensors
            qT = res.tile([128, 2, S], f16)
            kT = res.tile([128, 2, S], f16)
            v_sb = res.tile([128, ST, HPC, 65], f16)
            woT_sb = res.tile([128, 2, DIM], f16)
            maskD_in = res.tile([128, 128], f32)
            maskL_in = res.tile([128, 128], f32)
            maskD_sb = res.tile([128, 128], f32)
            maskLD_sb = res.tile([128, 2, 128], f32)
            ident_sb = res.tile([128, 128], f16)
            wq_sb = res.tile([128, KC, E], f16)
            wk_sb = res.tile([128, KC, E], f16)
            wv_sb = res.tile([128, KC, E], f16)
            A_sb = res.tile([128, S], f16)
            B_sb = res.tile([128, S], f16)
            perm_sb = res.tile([128, 128], f16)
            qraw = res.tile([128, 2, S], f16)
            kraw = res.tile([128, 2, S], f16)

            nc.sync.dma_start(out=woT_sb, in_=woT[:].rearrange("(c p) n -> p c n", p=128))
            nc.sync.dma_start(out=maskD_in, in_=maskD[:])
            nc.sync.dma_start(out=maskL_in, in_=maskL[:])
            nc.sync.dma_start(out=ident_sb, in_=ident[:])
            # launder the masks through DVE so the per-tile mask adds depend
            # on the DVE engine (program order, no sem) instead of DMA sems
            nc.vector.tensor_copy(maskD_sb, maskD_in)
            nc.vector.tensor_copy(maskLD_sb[:, 0, :], maskL_in)
            nc.vector.tensor_copy(maskLD_sb[:, 1, :], maskD_in)
            nc.any.memset(v_sb[:, :, :, 64:65], 1.0)

            for dram, sb in ((wqT, wq_sb), (wkT, wk_sb), (wvT, wv_sb)):
                nc.sync.dma_start(out=sb, in_=dram[:].rearrange("(c p) e -> p c e", p=128))
            nc.sync.dma_start(out=A_sb, in_=ropeA[:])
            nc.sync.dma_start(out=B_sb, in_=ropeB[:])
            nc.sync.dma_start(out=perm_sb, in_=permM[:])

            # ------------- phase 1: projections + RoPE, streamed by s -------------
            with tc.tile_pool(name="pp", bufs=2, space="PSUM") as pp:
              for sc in range(4):
                ssl = slice(sc * 512, (sc + 1) * 512)
                # q/k projections, transposed output [e, s]
                for dram, w_sb, raw in ((xqT, wq_sb, qraw), (xkT, wk_sb, kraw)):
                    xt = xp.tile([128, KC, 512], f16, tag="xt")
                    for kc in range(KC):
                        nc.sync.dma_start(
                            out=xt[:, kc, :], in_=dram[kc * 128 : (kc + 1) * 128, ssl]
                        )
                    for ec in range(2):
                        ps = pp.tile([128, 512], f32, tag="pp")
                        for kc in range(KC):
                            nc.tensor.matmul(
                                ps,
                                lhsT=w_sb[:, kc, ec * 128 : (ec + 1) * 128],
                                rhs=xt[:, kc, :],
                                start=(kc == 0),
                                stop=(kc == KC - 1),
                            )
                        nc.scalar.copy(raw[:, ec, ssl], ps)
                # v projection, natural output [s, e], into v_ext slots
                xt = xp.tile([128, KC, 512], f16, tag="xt")
                for kc in range(KC):
                    nc.sync.dma_start(
                        out=xt[:, kc, :], in_=xvT[kc * 128 : (kc + 1) * 128, ssl]
                    )
                for st4 in range(4):
                    sc16 = sc * 4 + st4
                    ps = pp.tile([128, E], f32, tag="ppv")
                    for kc in range(KC):
                        nc.tensor.matmul(
                            ps,
                            lhsT=xt[:, kc, st4 * 128 : (st4 + 1) * 128],
                            rhs=wv_sb[:, kc, :],
                            start=(kc == 0),
                            stop=(kc == KC - 1),
                        )
                    nc.scalar.copy(
                        v_sb[:, sc16, :, 0:64],
                        ps.rearrange("p (h d) -> p h d", h=HPC),
                    )
                # RoPE on this s-chunk: out = raw*A + permute(raw)*B
                for raw, out_sb in ((qraw, qT), (kraw, kT)):
                    for ec in range(2):
                        psh = pp.tile([128, 512], f32, tag="perm")
                        nc.tensor.matmul(
                            psh, lhsT=perm_sb, rhs=raw[:, ec, ssl],
                            start=True, stop=True,
                        )
                        t1 = tmpp.tile([128, 512], f16, tag="t1")
                        nc.gpsimd.tensor_mul(t1, raw[:, ec, ssl], A_sb[:, ssl])
                        t2 = tmpp.tile([128, 512], f16, tag="t2")
                        nc.vector.tensor_mul(t2, psh, B_sb[:, ssl])
                        nc.vector.tensor_add(out_sb[:, ec, ssl], t1, t2)

            # ---------------- phase 2: attention + out-proj ----------------
            with tc.tile_pool(name="pst", bufs=2, space="PSUM") as stp, \
                 tc.tile_pool(name="po", bufs=2, space="PSUM") as op, \
                 tc.tile_pool(name="ptr", bufs=1, space="PSUM") as trp, \
                 tc.tile_pool(name="py", bufs=1, space="PSUM") as yp:
                for t in range(ST):
                    c0 = max(0, t - 4)
                    ncv = t - c0 + 1
                    tsl = slice(t * 128, (t + 1) * 128)
                    attn_t = sb2.tile([128, HPC, 64], f16, tag="attn")
                    po = op.tile([128, HPC, 66], f32, tag="po")
                    for h in range(HPC):
                        ec, hh = h // 2, h % 2
                        psl = slice(hh * 64, (hh + 1) * 64)
                        pst = stp.tile([128, 5, 128], f32, tag="st")
                        for si, c in enumerate(range(c0, t + 1)):
                            nc.tensor.matmul(
                                pst[:, si, :],
                                lhsT=kT[psl, ec, c * 128 : (c + 1) * 128],
                                rhs=qT[psl, ec, tsl],
                                start=True,
                                stop=True,
                            )
                        if t >= 4:
                            # boundary chunks 0 and 4 masked in one strided op
                            nc.vector.tensor_add(
                                pst[:, 0:5:4, :], pst[:, 0:5:4, :], maskLD_sb
                            )
                        else:
                            nc.vector.tensor_add(
                                pst[:, ncv - 1, :], pst[:, ncv - 1, :], maskD_sb
                            )
                        pt = sb2.tile([128, 5, 128], f16, tag="pt", bufs=4)
                        nc.scalar.activation(
                            pt[:, 0:ncv, :], pst[:, 0:ncv, :], Exp, scale=0.125
                        )
                        for si, c in enumerate(range(c0, t + 1)):
                            nc.tensor.matmul(
                                po[:, h, 0:65],
                                lhsT=pt[:, si, :],
                                rhs=v_sb[:, c, h, :],
                                start=(si == 0),
                                stop=(si == ncv - 1),
                            )
                    rc = sb2.tile([128, HPC, 1], f32, tag="rc")
                    nc.vector.reciprocal(rc, po[:, :, 64:65])
                    nc.vector.tensor_mul(
                        attn_t, po[:, :, 0:64], rc.broadcast_to([128, HPC, 64])
                    )

                    attnT_t = sb2.tile([128, 2, 128], f16, tag="attnT")
                    attn_flat = attn_t.rearrange("p h d -> p (h d)")
                    for ec in range(2):
                        ptr = trp.tile([128, 128], f16, tag="tr")
                        nc.tensor.transpose(
                            ptr, attn_flat[:, ec * 128 : (ec + 1) * 128], ident_sb
                        )
                        nc.vector.tensor_copy(attnT_t[:, ec, :], ptr)
                    for nch in range(2):
                        py = yp.tile([128, 512], f32, tag="py")
                        for ec in range(2):
                            nc.tensor.matmul(
                                py,
                                lhsT=attnT_t[:, ec, :],
                                rhs=woT_sb[:, ec, nch * 512 : (nch + 1) * 512],
                                start=(ec == 0),
                                stop=(ec == 1),
                            )
                        y_sb = sb2.tile([128, 512], bf16, tag="ysb", bufs=3)
                        # alternate the PSUM->SBUF cast between ACT and DVE to
                        # balance engine load
                        if (t + nch) % 2 == 0:
                            nc.scalar.copy(y_sb, py)
                        else:
                            nc.vector.tensor_copy(y_sb, py)
                        nc.sync.dma_start(
                            out=y[tsl, nch * 512 : (nch + 1) * 512], in_=y_sb
                        )
    if do_compile:
        # Bacc pass pipeline: splits multi-sem waits into EventSemaphores
        # (HW allows 1 sync wait per instruction), register allocation, DCE.
        nc.compile()
    return nc


_CACHE = {}


def _get_nc():
    if "nc" not in _CACHE:
        _CACHE["nc"] = build_bass()
    return _CACHE["nc"]


def _in_maps(q, k, v, wq, wk, wv, wo):
    consts = _consts()
    perm = _head_perm()
    maps = []
    for c in range(N_CORES):
        b, g = c // 4, c % 4
        esl = slice(g * E, (g + 1) * E)
        wq_c = wq[esl][perm]
        wk_c = wk[esl][perm]
        m = {
            "xqT": np.ascontiguousarray(q[b].T),
            "xkT": np.ascontiguousarray(k[b].T),
            "xvT": np.ascontiguousarray(v[b].T),
            "wqT": np.ascontiguousarray(wq_c.T),
            "wkT": np.ascontiguousarray(wk_c.T),
            "wvT": np.ascontiguousarray(wv[esl].T),
            "woT": np.ascontiguousarray(wo[:, esl].T),
        }
        m.update(consts)
        maps.append(m)
    return maps


def kernel(q, k, v, wq, wk, wv, wo):
    q, k, v = (np.asarray(a, dtype=np.float16) for a in (q, k, v))
    wq, wk, wv, wo = (np.asarray(a, dtype=np.float16) for a in (wq, wk, wv, wo))
    from concourse.bass_utils import run_bass_kernel_spmd

    nc = _get_nc()
    maps = _in_maps(q, k, v, wq, wk, wv, wo)
    res = run_bass_kernel_spmd(nc, maps, core_ids=list(range(N_CORES)))
    out = np.zeros((B, S, DIM), dtype=np.float32)
    for c in range(N_CORES):
        out[c // 4] += np.asarray(res.results[c]["y"]).astype(np.float32)
    return out.astype(np.float16)
